# revision 1
# baseline (speedup 1.0000x reference)
"""Self-contained Trainium2 Bass kernel for GQA MultiHeadAttention with RoPE.

Problem: B=2, S=2048, D=1024, H=16 Q heads, KVH=4 KV heads, head_dim=64,
causal additive mask, f32.

Sharding: tensor-parallel over heads (TP=4: 4 Q heads + 1 KV head per shard)
x data-parallel over batch (DP=2) = 8 NeuronCores. Wo is sharded on its
input dim; the host sums the 4 partial outputs per batch element.
"""

import os
import sys

for _p in ("/opt/trn_rl_repo", "/root/.axon_site/_ro/trn_rl_repo"):
    if os.path.isdir(_p) and _p not in sys.path:
        sys.path.insert(0, _p)

import numpy as np
import ml_dtypes

import concourse.bacc as bacc
import concourse.bass as bass
import concourse.tile as tile
from concourse import mybir
from concourse.bass_utils import run_bass_kernel_spmd

F32 = mybir.dt.float32
F32R = mybir.dt.float32r
BF16 = mybir.dt.bfloat16
AF = mybir.ActivationFunctionType

H, KVH, HD = 16, 4, 64
B, S, D = 2, 2048, 1024
TP = 4                      # head-parallel ways
SCALE = HD ** -0.5
NEG = -1e9
NT = S // 128               # 16 kv tiles
NQB = S // 512              # 4 q blocks

PHASES = "D"                # profiling knob: stop after phase A/B/C/D


def _patch_act_tables():
    """Make Exp/Ln resolve only to natural_log_exp_and_others so the
    act-table-load pass emits one load instead of thrashing between the
    exp-only and ln-only sets."""
    from concourse.hw_specs import get_activation_tables
    t = get_activation_tables("gen3")
    for name, fns in t.items():
        if name != "natural_log_exp_and_others":
            fns.discard(AF.Exp)
            fns.discard(AF.Ln)


def _build_nc(causal: bool):
    _patch_act_tables()
    nc = bacc.Bacc()

    hT = nc.declare_dram_parameter("hT", [D, S], BF16, isOutput=False)
    cs64 = nc.declare_dram_parameter("cs64", [64, S], F32, isOutput=False)
    sn64 = nc.declare_dram_parameter("sn64", [64, S], F32, isOutput=False)
    wq = nc.declare_dram_parameter("wq", [D, 256], BF16, isOutput=False)
    wkv = nc.declare_dram_parameter("wkv", [D, 128], BF16, isOutput=False)
    wo = nc.declare_dram_parameter("wo", [256, D], F32R, isOutput=False)
    psigT = nc.declare_dram_parameter("psigT", [128, 128], F32R, isOutput=False)
    ident = nc.declare_dram_parameter("ident", [128, 128], F32R, isOutput=False)
    m01 = nc.declare_dram_parameter("m01", [128, 128], F32, isOutput=False)
    ones16 = nc.declare_dram_parameter("ones16", [128, 16], F32R, isOutput=False)
    outp = nc.declare_dram_parameter("out", [S, D], F32, isOutput=True)
    rscratch = nc.dram_tensor("rscratch", [4 * NQB, 512], F32)
    rscratch2 = nc.dram_tensor("rscratch2", [NQB * 2048], F32)

    with tile.TileContext(nc) as tc:
        with tc.tile_pool(name="hold", bufs=1) as hp:
            # weights first (small), then hidden in column halves so the
            # first projection chunk can start at ~half the load time
            wq_sb = hp.tile([128, 8, 256], BF16, name="wq_sb", tag="wq_sb")
            nc.sync.dma_start(out=wq_sb, in_=wq.rearrange("(c p) n -> p c n", p=128))
            wkv_sb = hp.tile([128, 8, 128], BF16, name="wkv_sb", tag="wkv_sb")
            nc.sync.dma_start(out=wkv_sb, in_=wkv.rearrange("(c p) n -> p c n", p=128))
            cosf_sb = hp.tile([128, S], F32, name="cosf_sb", tag="cosf_sb")
            sinf_sb = hp.tile([128, S], F32, name="sinf_sb", tag="sinf_sb")
            nc.sync.dma_start(out=cosf_sb[0:64, :], in_=cs64[:, :])
            nc.sync.dma_start(out=cosf_sb[64:128, :], in_=cs64[:, :])
            nc.sync.dma_start(out=sinf_sb[0:64, :], in_=sn64[:, :])
            nc.sync.dma_start(out=sinf_sb[64:128, :], in_=sn64[:, :])
            psig_sb = hp.tile([128, 128], F32R, name="psig_sb", tag="psig_sb")
            nc.sync.dma_start(out=psig_sb, in_=psigT[:, :])
            id_sb = hp.tile([128, 128], F32R, name="id_sb", tag="id_sb")
            nc.sync.dma_start(out=id_sb, in_=ident[:, :])
            m01_sb = hp.tile([128, 128], F32, name="m01_sb", tag="m01_sb")
            nc.sync.dma_start(out=m01_sb, in_=m01[:, :])
            wo_sb = hp.tile([128, 2, D], F32R, name="wo_sb", tag="wo_sb")
            nc.sync.dma_start(out=wo_sb,
                              in_=wo.rearrange("(c p) n -> p c n", p=128))

            ht_sb = [hp.tile([128, S], BF16, name=f"ht{c}", tag=f"ht{c}")
                     for c in range(8)]
            for half in range(2):
                hsl = slice(1024 * half, 1024 * half + 1024)
                for c in range(8):
                    nc.sync.dma_start(out=ht_sb[c][:, hsl],
                                      in_=hT[c * 128:(c + 1) * 128, hsl])

            qTs = [hp.tile([128, S], F32R, name=f"qT{p}", tag=f"qT{p}")
                   for p in range(2)]
            kT = hp.tile([128, S], F32R, name="kTt", tag="kTt")
            vsm = hp.tile([128, NT, 65], F32R, name="vsm", tag="vsm")
            ctxTs = [[hp.tile([128, 512], F32R, name=f"ctxT{c}_{q}",
                              tag=f"ctxT{c}_{q}") for q in range(NQB)]
                     for c in range(2)]

            # ones column (64) of vsm for the softmax denominator row
            nc.sync.dma_start(out=vsm[:, :, 64:65],
                              in_=ones16.rearrange("p (n o) -> p n o", o=1))

            # ---- Phases A-D share one PSUM budget via tag-sharing ----
            with tc.tile_pool(name="psS", bufs=1, space="PSUM") as psS, \
                 tc.tile_pool(name="psC", bufs=1, space="PSUM") as psC, \
                 tc.tile_pool(name="etp", bufs=1) as etp, \
                 tc.tile_pool(name="sbA", bufs=3) as sbA, \
                 tc.tile_pool(name="sbC", bufs=1) as sbC:

                # ---------------- Phase A: projections + rope ----------------
                def emit_q(pp):
                    for sc in range(4):
                        csl = slice(512 * sc, 512 * sc + 512)
                        ps_q = psS.tile([128, 1024], F32, name="ps_q",
                                        tag="ps_s", bufs=2)[:, 0:512]
                        for dc in range(8):
                            nc.tensor.matmul(
                                ps_q,
                                wq_sb[:, dc, 128 * pp:128 * pp + 128],
                                ht_sb[dc][:, csl],
                                start=(dc == 0), stop=(dc == 7))
                        qraw = sbA.tile([128, 512], F32R, name="qraw", tag="qraw")
                        nc.scalar.copy(qraw, ps_q)
                        ps_rot = psS.tile([128, 512], F32, name="ps_rot",
                                          tag="ps_d", bufs=2)
                        nc.tensor.matmul(ps_rot, psig_sb.bitcast(F32R),
                                         qraw.bitcast(F32R), start=True, stop=True)
                        dst = qTs[pp][:, csl]
                        nc.vector.tensor_mul(dst, qraw.bitcast(F32),
                                             cosf_sb[:, csl])
                        rtmp = sbA.tile([128, 512], F32, name="rtmp", tag="rtmp")
                        nc.vector.tensor_mul(rtmp, ps_rot, sinf_sb[:, csl])
                        nc.vector.tensor_add(dst, dst.bitcast(F32), rtmp)

                def emit_kv():
                    # K/V: kvT = [Wk|Wv].T @ h.T -> K rows 0:64, V rows 64:128
                    for sc in range(4):
                        csl = slice(512 * sc, 512 * sc + 512)
                        ps_kv = psS.tile([128, 1024], F32, name="ps_kv",
                                         tag="ps_s", bufs=2)[:, 0:512]
                        for dc in range(8):
                            nc.tensor.matmul(
                                ps_kv,
                                wkv_sb[:, dc, :],
                                ht_sb[dc][:, csl],
                                start=(dc == 0), stop=(dc == 7))
                        kvraw = sbA.tile([128, 512], F32R, name="kvraw",
                                         tag="kvraw")
                        nc.scalar.copy(kvraw, ps_kv)
                        # rope on K rows
                        ps_krot = psS.tile([128, 512], F32, name="ps_krot",
                                           tag="ps_d", bufs=2)[0:64, :]
                        nc.tensor.matmul(ps_krot,
                                         psig_sb[0:64, 0:64].bitcast(F32R),
                                         kvraw[0:64, :].bitcast(F32R),
                                         start=True, stop=True)
                        kdst = kT[0:64, csl]
                        nc.vector.tensor_mul(kdst, kvraw[0:64, :].bitcast(F32),
                                             cosf_sb[0:64, csl])
                        ktmp = sbA.tile([64, 512], F32, name="ktmp", tag="ktmp")
                        nc.vector.tensor_mul(ktmp, ps_krot, sinf_sb[0:64, csl])
                        nc.vector.tensor_add(kdst, kdst.bitcast(F32), ktmp)
                        # V: transpose each 128-seq tile into vsm (seq-major)
                        for tt in range(4):
                            ti = 4 * sc + tt
                            ps_v = psC.tile([128, 512], F32, name="ps_v",
                                            tag="ps_ctx", bufs=2)[:, 0:64]
                            nc.tensor.matmul(
                                ps_v.bitcast(F32R),
                                kvraw[64:128, 128 * tt:128 * tt + 128].bitcast(F32R),
                                id_sb[64:128, 0:64].bitcast(F32R),
                                start=True, stop=True, is_transpose=True)
                            nc.vector.tensor_copy(vsm[:, ti, 0:64], ps_v)
                    # duplicate roped K to partitions 64:128 so odd heads can
                    # use base-64 aligned operands (engines cannot cross
                    # partitions; DMA can)
                    nc.sync.dma_start(out=kT[64:128, :], in_=kT[0:64, :])

                def emit_phase_d(dq):
                    for qt in range(4 * dq, 4 * dq + 4):
                        for nb in range(2):
                            ps_o = psS.tile([128, 512], F32, name="ps_o",
                                            tag="ps_d", bufs=2)
                            for c in range(2):
                                ct = ctxTs[c][qt // 4]
                                col = 128 * (qt % 4)
                                nc.tensor.matmul(
                                    ps_o,
                                    ct[:, col:col + 128].bitcast(F32R),
                                    wo_sb[:, c, 512 * nb:512 * nb + 512].bitcast(F32R),
                                    start=(c == 0), stop=(c == 1))
                            ost = sbC.tile([128, 512], F32, name="ost",
                                           tag="ost", bufs=4)
                            if nb == 0:
                                nc.vector.tensor_copy(ost, ps_o)
                            else:
                                nc.scalar.copy(ost, ps_o)
                            nc.sync.dma_start(
                                out=outp[128 * qt:128 * qt + 128,
                                         512 * nb:512 * nb + 512],
                                in_=ost)

                def emit_bc(qb, sp, last=False):
                    # attention + normalization for one (q block, slot pair)
                    ctxu = sbC.tile([65, 1024], F32, name="ctxu", tag="ctxu",
                                    bufs=3)
                    for hh in range(2):
                        h = 2 * sp + hh
                        off = 64 * (h % 2)
                        pp = h // 2
                        ps_ctx = psC.tile([128, 512], F32, name="ps_ctx",
                                          tag="ps_ctx", bufs=2)
                        nki = (4 * qb + 4) if causal else NT
                        nfull = (4 * qb) if causal else NT
                        # software-pipelined tile units: emit the NEXT unit's
                        # scores matmuls before this unit's ctx matmuls so PE
                        # never waits on the exp
                        units = []

                        def mk_pair(kp, _off=off, _pp=pp, _qb=qb, _nki=nki,
                                    _ps_ctx=ps_ctx):
                            box = {}

                            def s():
                                ps_s = psS.tile([128, 1024], F32, name="ps_s",
                                                tag="ps_s", bufs=2)
                                for jj in range(2):
                                    ki = kp + jj
                                    nc.tensor.matmul(
                                        ps_s[:, 512 * jj:512 * jj + 512],
                                        kT[_off:_off + 64,
                                           128 * ki:128 * ki + 128].bitcast(F32R),
                                        qTs[_pp][_off:_off + 64,
                                            512 * _qb:512 * _qb + 512].bitcast(F32R),
                                        start=True, stop=True)
                                box["ps"] = ps_s

                            def ec():
                                et = etp.tile([128, 1024], F32R, name="et",
                                              tag="et", bufs=4)
                                nc.scalar.activation(et, box["ps"], AF.Exp,
                                                     scale=SCALE)
                                for jj in range(2):
                                    ki = kp + jj
                                    nc.tensor.matmul(
                                        _ps_ctx[0:65, :],
                                        vsm[:, ki, 0:65].bitcast(F32R),
                                        et[:, 512 * jj:512 * jj + 512].bitcast(F32R),
                                        start=(ki == 0), stop=(ki == _nki - 1))
                            return (s, ec)

                        def mk_diag(j, _off=off, _pp=pp, _qb=qb, _nki=nki,
                                    _ps_ctx=ps_ctx):
                            box = {}
                            ki = 4 * _qb + j
                            soff, span = 128 * j, 512 - 128 * j

                            def s():
                                ps_d = psS.tile([128, 512], F32, name="ps_d",
                                                tag="ps_d", bufs=2)
                                nc.tensor.matmul(
                                    ps_d[:, :span],
                                    kT[_off:_off + 64,
                                       128 * ki:128 * ki + 128].bitcast(F32R),
                                    qTs[_pp][_off:_off + 64,
                                        512 * _qb + soff:512 * (_qb + 1)].bitcast(F32R),
                                    start=True, stop=True)
                                box["ps"] = ps_d

                            def ec():
                                etd = etp.tile([128, 512], F32R, name="etd",
                                               tag="etd", bufs=4)
                                nc.scalar.activation(etd[:, :span],
                                                     box["ps"][:, :span],
                                                     AF.Exp, scale=SCALE)
                                ceng = nc.gpsimd if j % 2 == 0 else nc.vector
                                ceng.tensor_mul(etd[:, :128],
                                                etd[:, :128].bitcast(F32),
                                                m01_sb)
                                nc.tensor.matmul(
                                    _ps_ctx[0:65, soff:512],
                                    vsm[:, ki, 0:65].bitcast(F32R),
                                    etd[:, :span].bitcast(F32R),
                                    start=(ki == 0), stop=(ki == _nki - 1))
                            return (s, ec)

                        for kp in range(0, nfull, 2):
                            units.append(mk_pair(kp))
                        if causal:
                            for j in range(4):
                                units.append(mk_diag(j))
                        if units:
                            units[0][0]()
                        for i in range(len(units)):
                            if i + 1 < len(units):
                                units[i + 1][0]()
                            units[i][1]()
                        # evict unnormalized ctx + rowsum, freeing psum
                        nc.vector.tensor_copy(
                            ctxu[0:65, 512 * hh:512 * hh + 512],
                            ps_ctx[0:65, :])
                    if PHASES == "B":
                        return
                    # ---- phase C: batched reciprocal of the 2 rowsum rows ----
                    sbase = 2048 * qb + 1024 * sp
                    if last:
                        # tail fast path: ln/exp directly on the (idle) ACT at
                        # 1-partition width, skipping the [128,8] reshape hops
                        nc.scalar.activation(ctxu[64:65, :], ctxu[64:65, :],
                                             AF.Ln)
                        nc.scalar.activation(ctxu[64:65, :], ctxu[64:65, :],
                                             AF.Exp, scale=-1.0)
                        s_ap = rscratch2[sbase:sbase + 1024]
                        nc.sync.dma_start(
                            out=bass.AP(tensor=s_ap.tensor, offset=s_ap.offset,
                                        ap=[[1, 1], [1, 1024]]),
                            in_=ctxu[64:65, :])
                    else:
                        for hh in range(2):
                            slot = 4 * qb + 2 * sp + hh
                            nc.sync.dma_start(
                                out=rscratch[slot, :],
                                in_=ctxu[64:65, 512 * hh:512 * hh + 512])
                        rs = sbC.tile([128, 8], F32, name="rs", tag="rs", bufs=2)
                        g_ap = rscratch[4 * qb + 2 * sp]
                        nc.sync.dma_start(
                            out=rs, in_=bass.AP(tensor=g_ap.tensor,
                                                offset=g_ap.offset,
                                                ap=[[8, 128], [1, 8]]))
                        nc.scalar.activation(rs, rs, AF.Ln)
                        nc.scalar.activation(rs, rs, AF.Exp, scale=-1.0)
                        s_ap = rscratch2[sbase:sbase + 1024]
                        nc.sync.dma_start(
                            out=bass.AP(tensor=s_ap.tensor, offset=s_ap.offset,
                                        ap=[[8, 128], [1, 8]]), in_=rs)
                    if sp == 1:
                        # move the cross-partition hop off the critical path:
                        # copy UNNORMALIZED ctx to base 64 now (depends only
                        # on the psum evict), normalize in place once the
                        # reciprocal arrives
                        for hh in range(2):
                            nc.sync.dma_start(
                                out=ctxTs[hh][qb][64:128, :],
                                in_=ctxu[0:64,
                                         512 * hh:512 * hh + 512].bitcast(F32R))
                    for hh in range(2):
                        ct = ctxTs[hh][qb]
                        if sp == 0:
                            rb = sbC.tile([64, 512], F32, name="rb", tag="rb",
                                          bufs=4)
                            r_ap = rscratch2[sbase + 512 * hh:
                                             sbase + 512 * (hh + 1)]
                            nc.gpsimd.dma_start(
                                out=rb, in_=bass.AP(tensor=r_ap.tensor,
                                                    offset=r_ap.offset,
                                                    ap=[[0, 64], [1, 512]]))
                            nc.vector.tensor_mul(
                                ct[0:64, :],
                                ctxu[0:64, 512 * hh:512 * hh + 512], rb)
                        else:
                            rb = sbC.tile([128, 512], F32, name="rbw",
                                          tag="rbw", bufs=4)
                            r_ap = rscratch2[sbase + 512 * hh:
                                             sbase + 512 * (hh + 1)]
                            nc.gpsimd.dma_start(
                                out=rb, in_=bass.AP(tensor=r_ap.tensor,
                                                    offset=r_ap.offset,
                                                    ap=[[0, 128], [1, 512]]))
                            nc.vector.tensor_mul(
                                ct[64:128, :], ct[64:128, :].bitcast(F32),
                                rb[64:128, :])

                # ---- global emission order: overlap phase A with qb=0 ----
                emit_kv()
                emit_q(0)
                if PHASES == "A":
                    emit_q(1)
                else:
                    emit_bc(0, 0)
                    emit_q(1)
                    emit_bc(0, 1)
                    for qb in range(1, NQB):
                        emit_bc(qb, 0, last=(qb == NQB - 1))
                        emit_bc(qb, 1, last=(qb == NQB - 1))
                        if PHASES == "D":
                            emit_phase_d(qb - 1)
                    if PHASES == "D":
                        emit_phase_d(NQB - 1)

    nc.compile()
    return nc


_NC_CACHE = {}


def _get_nc(causal: bool):
    if causal not in _NC_CACHE:
        _NC_CACHE[causal] = _build_nc(causal)
    return _NC_CACHE[causal]


def _host_consts():
    p = np.zeros((128, 128), np.float32)
    idx = np.arange(0, 128, 2)
    p[idx, idx + 1] = -1.0
    p[idx + 1, idx] = 1.0
    psigT = np.ascontiguousarray(p.T)
    ident = np.eye(128, dtype=np.float32)
    ident[64:128, 0:64] = np.eye(64, dtype=np.float32)
    m01 = (np.arange(128)[None, :] >= np.arange(128)[:, None]).astype(np.float32)
    return psigT, ident, m01


def _numpy_reference(hidden_states, cos, sin, attention_mask, Wq, Wk, Wv, Wo):
    """Generic-mask fallback, pure numpy port of the reference."""
    GROUPS = H // KVH

    def rope(x, c, s):
        c = c[:, None, :, :]
        s = s[:, None, :, :]
        x1, x2 = x[..., ::2], x[..., 1::2]
        xr = np.stack([x1 * c - x2 * s, x1 * s + x2 * c], axis=-1)
        return xr.reshape(x.shape)

    b, sq, d = hidden_states.shape
    q = (hidden_states @ Wq).reshape(b, sq, H, HD).transpose(0, 2, 1, 3)
    k = (hidden_states @ Wk).reshape(b, sq, KVH, HD).transpose(0, 2, 1, 3)
    v = (hidden_states @ Wv).reshape(b, sq, KVH, HD).transpose(0, 2, 1, 3)
    q = rope(q, cos, sin)
    k = rope(k, cos, sin)
    k = np.repeat(k, GROUPS, axis=1)
    v = np.repeat(v, GROUPS, axis=1)
    out = np.zeros((b, sq, d), np.float32)
    for bi in range(b):
        for hi in range(H):
            sc = (q[bi, hi] @ k[bi, hi].T) * SCALE + attention_mask[0, 0]
            sc = sc - sc.max(axis=-1, keepdims=True)
            e = np.exp(sc)
            pr = e / e.sum(axis=-1, keepdims=True)
            ctx = pr @ v[bi, hi]
            out[bi] += ctx @ Wo[hi * HD:(hi + 1) * HD]
    return out


def kernel(**inputs) -> np.ndarray:
    hs = np.asarray(inputs["hidden_states"], np.float32)
    cos = np.asarray(inputs["cos"], np.float32)
    sin = np.asarray(inputs["sin"], np.float32)
    mask = np.asarray(inputs["attention_mask"], np.float32)
    Wq = np.asarray(inputs["Wq"], np.float32)
    Wk = np.asarray(inputs["Wk"], np.float32)
    Wv = np.asarray(inputs["Wv"], np.float32)
    Wo = np.asarray(inputs["Wo"], np.float32)

    m = mask.reshape(S, S)
    tril = np.tril(np.ones((S, S), dtype=bool))
    causal_ref = np.where(tril, np.float32(0.0), np.float32(NEG))
    if np.array_equal(m, causal_ref):
        causal = True
    elif not m.any():
        causal = False
    else:
        return _numpy_reference(hs, cos, sin, mask, Wq, Wk, Wv, Wo)

    nc = _get_nc(causal)
    psigT, ident, m01 = _host_consts()
    chan_half = (np.arange(64) // 2)

    in_maps = []
    for core in range(8):
        b, t = core // TP, core % TP
        hT = np.ascontiguousarray(hs[b].T).astype(ml_dtypes.bfloat16)
        cs64v = np.ascontiguousarray(cos[b].T[chan_half, :])
        sn64v = np.ascontiguousarray(sin[b].T[chan_half, :])
        wq_s = np.ascontiguousarray(
            Wq[:, t * 256:(t + 1) * 256]).astype(ml_dtypes.bfloat16)
        wkv_s = np.ascontiguousarray(
            np.concatenate([Wk[:, t * 64:(t + 1) * 64],
                            Wv[:, t * 64:(t + 1) * 64]],
                           axis=1)).astype(ml_dtypes.bfloat16)
        wo_s = Wo[t * 256:(t + 1) * 256]
        # ctxT channel order per chunk: c0 = [h0|h2], c1 = [h1|h3]
        wo_p = np.ascontiguousarray(
            np.concatenate([wo_s[0:64], wo_s[128:192],
                            wo_s[64:128], wo_s[192:256]], axis=0))
        in_maps.append({
            "hT": hT, "cs64": cs64v, "sn64": sn64v,
            "wq": wq_s, "wkv": wkv_s, "wo": wo_p,
            "psigT": psigT, "ident": ident, "m01": m01,
            "ones16": np.ones((128, 16), np.float32),
        })

    res = run_bass_kernel_spmd(nc, in_maps, core_ids=list(range(8)))
    out = np.zeros((B, S, D), np.float32)
    for core in range(8):
        out[core // TP] += res.results[core]["out"]
    return out

